# revision 10
# baseline (speedup 1.0000x reference)
"""ALBERT attention (B=2, S=2048, D=1024, H=16, K=64) on 8 TRN2 NeuronCores.

Sharding: core c = (b, g) with b = c // 4 (batch), g = c % 4 (head group of 4
heads). Each core computes output[b, :, 4g:4g+4, :] — outputs are disjoint, so
no collectives are needed.

Host-side prep: x is shipped transposed ([D, S], bf16); keys with
attention_mask == 0 are compacted away (they contribute exactly 0), padded to
a 128 multiple. Because of the compaction, only the LAST key tile contains
masked (padding) keys, so only its exp() needs the additive-mask bias.

Per-core pipeline (ScalarE exp is the roofline: ~64 ACTs x ~1.1us):
  - inputs stream in d-chunk-granular DMAs so the Q/K projections pipeline
    behind the HBM transfers; a dummy-matmul warmup keeps the PE HAM
    clock-gate at 8/8 before the real work lands.
  - projections, weight-stationary, bf16: QT [2-head 128, S] and per-pair
    KT [128, T] (logits contract 64 rows via tile_position), V computed
    DIRECTLY in [t, hk] layout (xt chunks stationary, wv moving) so no PE
    transpose pass is needed.
  - attention, head-sequential, f-half split (PSUM: lt 2x[128,1024] double
    buffered + cacc [65,1024] + 2 proj banks = 8): per (head, fhalf, ttile):
    logits LT [128 keys, 1024 f] (K=64 contraction), ScalarE
    ET = exp(0.125*LT [+ mask on last tile]) in bf16, context
    Cacc[65, 1024] += [1|V]^T @ ET (row 0 = softmax denominators).
    Remaining projections are issued below the attention stream so the Tile
    scheduler slots them into PE gaps while ACT (exp) is the bottleneck.
  - output ships UNNORMALIZED: out[h] = [65, S] f32 (row 0 = denom,
    rows 1..65 = C^T). Host divides, transposes, and adds bv (exact since
    probs sum to 1).
"""

import ml_dtypes
import numpy as np

import concourse.bass as bass
import concourse.tile as tile
from concourse import bacc, mybir
from concourse.bass_utils import run_bass_kernel_spmd

F32 = mybir.dt.float32
BF16 = mybir.dt.bfloat16

B, S, D, H, K = 2, 2048, 1024, 16, 64
NH = 4            # heads per core
HK = NH * K       # 256
NCORES = 8
DC = D // 128     # 8 contraction chunks
NEG = -10000.0
FH = 1024         # f-half width


def build_nc(t_tiles: int):
    """Per-core Bass graph. t_tiles = number of 128-row key tiles after
    host-side compaction of masked-out keys."""
    T = t_tiles * 128
    tchunks = [(c, min(512, T - c)) for c in range(0, T, 512)]

    nc = bacc.Bacc("TRN2", target_bir_lowering=False, debug=False,
                   num_devices=NCORES)

    xf_d = nc.dram_tensor("xf", [D, S], BF16, kind="ExternalInput").ap()
    xt_d = nc.dram_tensor("xt", [D, T], BF16, kind="ExternalInput").ap()
    wq_d = nc.dram_tensor("wq", [D, HK], BF16, kind="ExternalInput").ap()
    wk_d = nc.dram_tensor("wk", [D, HK], BF16, kind="ExternalInput").ap()
    wv_d = nc.dram_tensor("wv", [D, HK], BF16, kind="ExternalInput").ap()
    # bias columns: [bq0 bq1 bk0 bk1] (hk-tile halves of bq / bk)
    bias_d = nc.dram_tensor("bias", [128, 4], F32, kind="ExternalInput").ap()
    # additive key mask for the LAST key tile only (all other tiles are
    # fully unmasked after compaction)
    mask_d = nc.dram_tensor("mask", [128, 1], F32, kind="ExternalInput").ap()
    # unnormalized: per head, row 0 = softmax denominators, rows 1..64 = C^T
    out_d = nc.dram_tensor("out", [NH, K + 1, S], F32,
                           kind="ExternalOutput").ap()

    with tile.TileContext(nc) as tc:
        with (
            tc.sbuf_pool(name="const", bufs=1) as const_pool,
            tc.sbuf_pool(name="persist", bufs=1) as persist_pool,
            tc.psum_pool(name="proj", bufs=2) as proj_pool,
            tc.psum_pool(name="cacc", bufs=1) as cacc_pool,
            tc.sbuf_pool(name="et", bufs=t_tiles + 2) as et_pool,
            tc.sbuf_pool(name="ct", bufs=4) as ct_pool,
        ):
            bias_sb = const_pool.tile([128, 4], F32)
            mask_sb = const_pool.tile([128, 1], F32)
            warm_sb = const_pool.tile([128, 512], BF16)

            # big input tiles; views expose [p, chunk, col]
            xf_sb = persist_pool.tile([128, DC * S], BF16, name="xf")
            xt_sb = persist_pool.tile([128, DC * T], BF16, name="xt")
            wq_sb = persist_pool.tile([128, DC * HK], BF16, name="wq")
            wk_sb = persist_pool.tile([128, DC * HK], BF16, name="wk")
            wv_sb = persist_pool.tile([128, DC * HK], BF16, name="wv")
            xf_v = xf_sb.rearrange("p (c s) -> p c s", s=S)
            xt_v = xt_sb.rearrange("p (c s) -> p c s", s=T)
            wq_v = wq_sb.rearrange("p (c s) -> p c s", s=HK)
            wk_v = wk_sb.rearrange("p (c s) -> p c s", s=HK)
            wv_v = wv_sb.rearrange("p (c s) -> p c s", s=HK)

            qt_sb = [persist_pool.tile([128, S], BF16, name=f"qt{i}")
                     for i in range(2)]
            kt_sb = [persist_pool.tile([128, T], BF16, name=f"kt{i}")
                     for i in range(2)]
            # V with a leading ones column per head: [1|V_h0|1|V_h1|...]
            v_sb = [persist_pool.tile([128, NH * 65], BF16, name=f"v{i}")
                    for i in range(t_tiles)]
            nc.vector.memset(warm_sb[:], 0.0)
            for i in range(t_tiles):
                nc.vector.memset(
                    v_sb[i].rearrange("p (h c) -> p h c", c=65)[:, :, 0:1],
                    1.0)

            # ---------------- input DMAs ----------------
            # weights first (small; unblock the d-loops), then interleaved
            # xt / xf-fh0 d-chunks so projections stream behind HBM.
            xt_src = xt_d.rearrange("(c p) s -> p c s", p=128)
            xf_src = xf_d.rearrange("(c p) s -> p c s", p=128)
            nc.scalar.dma_start(
                wq_sb.rearrange("p (c s) -> p c s", s=HK),
                wq_d.rearrange("(c p) s -> p c s", p=128))
            nc.scalar.dma_start(
                wk_sb.rearrange("p (c s) -> p c s", s=HK),
                wk_d.rearrange("(c p) s -> p c s", p=128))
            for d in range(DC):
                nc.sync.dma_start(xt_v[:, d, :], xt_src[:, d, :])
                nc.gpsimd.dma_start(xf_v[:, d, 0:FH], xf_src[:, d, 0:FH])
            nc.scalar.dma_start(
                wv_sb.rearrange("p (c s) -> p c s", s=HK),
                wv_d.rearrange("(c p) s -> p c s", p=128))
            nc.scalar.dma_start(bias_sb[:], bias_d[:])
            nc.scalar.dma_start(mask_sb[:], mask_d[:])
            # xf f-half 1 (only needed once attention on fh0 is running)
            nc.gpsimd.dma_start(xf_v[:, :, FH:S], xf_src[:, :, FH:S])

            def q_proj(hk, fh, pool, drain_eng):
                """QT[hk][:, fh*FH:+FH] <- sum_d wq_d.T @ xf_d, + bq."""
                ps = [pool.tile([128, 512], F32, tag="pp",
                                name=f"qp{hk}_{fh}_{s}")
                      for s in range(2)]
                for d in range(DC):
                    lhs = wq_v[:, d, 128 * hk:128 * (hk + 1)]
                    for s in range(2):
                        c0 = fh * FH + 512 * s
                        nc.tensor.matmul(ps[s][:], lhs,
                                         xf_v[:, d, c0:c0 + 512],
                                         start=(d == 0), stop=(d == DC - 1))
                for s in range(2):
                    c0 = fh * FH + 512 * s
                    if drain_eng == "scalar":
                        nc.scalar.add(qt_sb[hk][:, c0:c0 + 512], ps[s][:],
                                      bias_sb[:, hk:hk + 1])
                    else:
                        nc.vector.tensor_scalar_add(
                            qt_sb[hk][:, c0:c0 + 512], ps[s][:],
                            bias_sb[:, hk:hk + 1])

            def k_chunk(hk, s, pool, drain_eng, interleave=None):
                """One T-chunk of KT[hk] <- sum_d wk_d.T @ xt_d, + bk.
                interleave: optional list of per-d callbacks (for pairing
                with another chunk's matmuls in issue order)."""
                c0, w = tchunks[s]
                pst = pool.tile([128, w], F32, tag="pp", name=f"kp{hk}_{s}")
                for d in range(DC):
                    nc.tensor.matmul(
                        pst[:], wk_v[:, d, 128 * hk:128 * (hk + 1)],
                        xt_v[:, d, c0:c0 + w],
                        start=(d == 0), stop=(d == DC - 1))
                    if interleave:
                        interleave[d]()
                if drain_eng == "scalar":
                    nc.scalar.add(kt_sb[hk][:, c0:c0 + w], pst[:],
                                  bias_sb[:, 2 + hk:3 + hk])
                else:
                    nc.vector.tensor_scalar_add(
                        kt_sb[hk][:, c0:c0 + w], pst[:],
                        bias_sb[:, 2 + hk:3 + hk])

            def v_proj(t):
                """v_sb[t][:, h*65+1 : h*65+65] <- (xt tile t).T @ wv.
                bv is added on the host (exact: probs sum to 1)."""
                ps = proj_pool.tile([128, HK], F32, tag="pp", name=f"vp{t}")
                for d in range(DC):
                    nc.tensor.matmul(ps[:],
                                     xt_v[:, d, 128 * t:128 * (t + 1)],
                                     wv_v[:, d, :],
                                     start=(d == 0), stop=(d == DC - 1))
                nc.vector.tensor_copy(
                    v_sb[t].rearrange("p (h c) -> p h c", c=65)[:, :, 1:65],
                    ps.rearrange("p (h c) -> p h c", c=64)[:, :, :])

            def attention(h, fh, lt_pool, defer_ctx_from=0):
                """Logits+exp chain for (h, fh). Context matmuls for
                t >= defer_ctx_from are returned as closures so callers can
                issue them AFTER the v_proj writes they depend on (Tile
                dependencies follow program order)."""
                hk, zo = h // 2, 64 * (h % 2)
                cacc = cacc_pool.tile([K + 1, FH], F32, tag="cacc",
                                      name=f"cacc{h}_{fh}")
                deferred = []
                for t in range(t_tiles):
                    lt = lt_pool.tile([128, FH], F32, tag="lt",
                                      name=f"lt{h}_{fh}_{t}")
                    for s in range(2):
                        c0 = fh * FH + 512 * s
                        nc.tensor.matmul(
                            lt[:, 512 * s:512 * (s + 1)],
                            kt_sb[hk][zo:zo + 64, 128 * t:128 * (t + 1)],
                            qt_sb[hk][zo:zo + 64, c0:c0 + 512],
                            start=True, stop=True)
                    et = et_pool.tile([128, FH], BF16, tag="et",
                                      name=f"et{h}_{fh}_{t}")
                    nc.scalar.activation(
                        et[:], lt[:], mybir.ActivationFunctionType.Exp,
                        bias=(mask_sb[:, 0:1] if t == t_tiles - 1 else 0.0),
                        scale=0.125)

                    def ctx(t=t, et=et):
                        for s in range(2):
                            nc.tensor.matmul(
                                cacc[:, 512 * s:512 * (s + 1)],
                                v_sb[t][:, 65 * h:65 * (h + 1)],
                                et[:, 512 * s:512 * (s + 1)],
                                start=(t == 0), stop=(t == t_tiles - 1),
                                skip_group_check=True)

                    if t >= defer_ctx_from:
                        deferred.append(ctx)
                    else:
                        ctx()

                def tail():
                    # drain + ship in halves so the final DMA overlaps copy
                    for s in range(2):
                        ct = ct_pool.tile([K + 1, 512], F32, tag="ct",
                                          name=f"ct{h}_{fh}_{s}")
                        nc.vector.tensor_copy(ct[:],
                                              cacc[:, 512 * s:512 * (s + 1)])
                        nc.sync.dma_start(
                            out_d[h][:, fh * FH + 512 * s:
                                     fh * FH + 512 * (s + 1)],
                            ct[:])

                deferred.append(tail)
                return deferred

            # ---------------- schedule ----------------
            # PE warmup: keep the HAM clock-gate hot until real work lands.
            with tc.psum_pool(name="warm", bufs=1) as warm_pool:
                wps = warm_pool.tile([128, 512], F32, tag="wp", name="warm")
                for i in range(22):
                    nc.tensor.matmul(wps[:], warm_sb[:, 0:128], warm_sb[:],
                                     start=True, stop=True)

            # critical path: Q(pair0, fh0) and K(pair0) interleaved d-wise
            # in a dedicated 4-bank pool that closes before lt opens.
            with tc.psum_pool(name="early", bufs=4) as early_pool:
                q_ps = [early_pool.tile([128, 512], F32, tag="pp",
                                        name=f"qp0_0_{s}")
                        for s in range(2)]

                def qcb(d):
                    lhs = wq_v[:, d, 0:128]
                    for s in range(2):
                        nc.tensor.matmul(
                            q_ps[s][:], lhs,
                            xf_v[:, d, 512 * s:512 * (s + 1)],
                            start=(d == 0), stop=(d == DC - 1))

                qcbs = [(lambda d=d: qcb(d)) for d in range(DC)]
                k_chunk(0, 0, early_pool, "scalar", interleave=qcbs)
                for s in range(2):
                    nc.vector.tensor_scalar_add(
                        qt_sb[0][:, 512 * s:512 * (s + 1)], q_ps[s][:],
                        bias_sb[:, 0:1])
                if len(tchunks) > 1:
                    k_chunk(0, 1, early_pool, "scalar")

            with tc.psum_pool(name="lt", bufs=2) as lt_pool:
                for s in range(2, len(tchunks)):
                    k_chunk(0, s, proj_pool, "scalar")
                v_proj(0)
                # h0/fh0: logits+exp start immediately; its contexts for
                # t>=1 are deferred until after the v_proj writes.
                pend = attention(0, 0, lt_pool, defer_ctx_from=1)
                for t in range(1, t_tiles):
                    v_proj(t)
                for op in pend:
                    op()
                for op in attention(1, 0, lt_pool):
                    op()
                for s in range(len(tchunks)):
                    k_chunk(1, s, proj_pool, "vector")
                q_proj(1, 0, proj_pool, "vector")
                for op in attention(2, 0, lt_pool):
                    op()
                q_proj(0, 1, proj_pool, "vector")
                for op in attention(3, 0, lt_pool):
                    op()
                q_proj(1, 1, proj_pool, "vector")
                for op in attention(0, 1, lt_pool):
                    op()
                for op in attention(1, 1, lt_pool):
                    op()
                for op in attention(2, 1, lt_pool):
                    op()
                for op in attention(3, 1, lt_pool):
                    op()

    nc.compile()
    return nc


_NC_CACHE = {}


def _get_nc(t_tiles: int):
    if t_tiles not in _NC_CACHE:
        _NC_CACHE[t_tiles] = build_nc(t_tiles)
    return _NC_CACHE[t_tiles]


def kernel(from_tensor, to_tensor, attention_mask, Wq, bq, Wk, bk, Wv, bv):
    from_tensor = np.asarray(from_tensor, dtype=np.float32)
    to_tensor = np.asarray(to_tensor, dtype=np.float32)
    attention_mask = np.asarray(attention_mask)
    Wq = np.asarray(Wq, dtype=np.float32)
    Wk = np.asarray(Wk, dtype=np.float32)
    Wv = np.asarray(Wv, dtype=np.float32)
    bq = np.asarray(bq, dtype=np.float32)
    bk = np.asarray(bk, dtype=np.float32)
    bv = np.asarray(bv, dtype=np.float32)

    # compact away masked-out keys (they contribute exactly 0 to the
    # context); pad to a 128 multiple and re-mask the padding tail.
    mask_np = attention_mask.astype(np.int32)
    idxs = [np.nonzero(mask_np[b])[0] for b in range(B)]
    t_eff = max(1, max(len(ix) for ix in idxs))
    T_pad = min(S, ((t_eff + 127) // 128) * 128)
    t_tiles = T_pad // 128
    nc = _get_nc(t_tiles)

    xt_c = np.zeros((B, D, T_pad), dtype=np.float32)
    maskadd = np.full((B, T_pad), NEG, dtype=np.float32)
    for b in range(B):
        ix = idxs[b]
        xt_c[b, :, :len(ix)] = to_tensor[b].T[:, ix]
        maskadd[b, :len(ix)] = 0.0

    in_maps = []
    for c in range(NCORES):
        b, g = c // 4, c % 4
        hs = slice(NH * g, NH * (g + 1))
        wq = np.ascontiguousarray(Wq[:, hs, :].reshape(D, HK))
        wk = np.ascontiguousarray(Wk[:, hs, :].reshape(D, HK))
        wv = np.ascontiguousarray(Wv[:, hs, :].reshape(D, HK))
        bias = np.stack([
            bq[hs].reshape(HK)[:128], bq[hs].reshape(HK)[128:],
            bk[hs].reshape(HK)[:128], bk[hs].reshape(HK)[128:],
        ], axis=1)
        in_maps.append({
            "xf": np.ascontiguousarray(from_tensor[b].T
                                       .astype(ml_dtypes.bfloat16)),
            "xt": np.ascontiguousarray(xt_c[b].astype(ml_dtypes.bfloat16)),
            "wq": wq.astype(ml_dtypes.bfloat16),
            "wk": wk.astype(ml_dtypes.bfloat16),
            "wv": wv.astype(ml_dtypes.bfloat16),
            "bias": np.ascontiguousarray(bias),
            "mask": np.ascontiguousarray(
                maskadd[b][(t_tiles - 1) * 128:].reshape(128, 1)),
        })

    global _LAST_IN_MAPS, _LAST_T_TILES
    _LAST_IN_MAPS = in_maps
    _LAST_T_TILES = t_tiles
    try:
        res = run_bass_kernel_spmd(nc, in_maps, core_ids=list(range(NCORES)))
    except Exception:
        # the axon terminal occasionally reports the device unrecoverable;
        # a reset + retry clears it
        try:
            import ctypes

            lib = ctypes.CDLL("/opt/axon/libaxon_pjrt.so")
            lib.axon_reset.restype = ctypes.c_int64
            lib.axon_reset()
        except Exception:
            pass
        res = run_bass_kernel_spmd(nc, in_maps, core_ids=list(range(NCORES)))

    out = np.empty((B, S, H, K), dtype=np.float32)
    for c in range(NCORES):
        b, g = c // 4, c % 4
        o = res.results[c]["out"]          # [NH, 65, S]
        ctx = o[:, 1:, :] / o[:, 0:1, :]   # normalize by denominators
        # [NH, K, S] -> [S, NH, K], plus bv
        out[b, :, NH * g:NH * (g + 1), :] = \
            ctx.transpose(2, 0, 1) + bv[NH * g:NH * (g + 1)][None]
    return out


# revision 11
# speedup vs baseline: 1.0480x; 1.0480x over previous
"""ALBERT attention (B=2, S=2048, D=1024, H=16, K=64) on 8 TRN2 NeuronCores.

Sharding: core c = (b, g) with b = c // 4 (batch), g = c % 4 (head group of 4
heads). Each core computes output[b, :, 4g:4g+4, :] — outputs are disjoint, so
no collectives are needed.

Host-side prep: x is shipped transposed ([D, S], bf16); keys with
attention_mask == 0 are compacted away (they contribute exactly 0), padded to
a 128 multiple. Because of the compaction, only the LAST key tile contains
masked (padding) keys, so only its exp() needs the additive-mask bias.

Per-core pipeline (ScalarE exp is the roofline: 64 ACTs x ~1.1us):
  - inputs stream in per-d-chunk DMAs into SEPARATE tiles so each consumer
    waits on exactly its own transfer; a dummy-matmul warmup keeps the PE
    HAM clock-gate at 8/8 until the real work lands.
  - projections, weight-stationary, bf16: QT [2-head 128, S] per pair and
    KT [128, T] per pair; V computed DIRECTLY in [t, hk] layout (xt chunks
    stationary, wv moving) so no PE transpose pass is needed.
  - attention runs per (head-PAIR, f-quarter 512): the two heads' logits
    matmuls contract 64 rows each at tile_position (0,0) / (64,0) so the PE
    runs them CONCURRENTLY into one lt [128, 1024] tile ([A | B]); a single
    exp ACT covers both; per-head contexts Cacc[65, 512] += [1|V]^T @ ET
    (row 0 = softmax denominators). PSUM: lt 2x2 + cacc 2x1 + proj 2 = 8.
    Remaining projections are issued below the attention stream to fill PE
    gaps while ACT (exp) is the bottleneck.
  - output ships UNNORMALIZED: out[h] = [65, S] f32 (row 0 = denom,
    rows 1..65 = C^T). Host divides, transposes, and adds bv (exact since
    probs sum to 1).
"""

import ml_dtypes
import numpy as np

import concourse.bass as bass
import concourse.tile as tile
from concourse import bacc, mybir
from concourse.bass_utils import run_bass_kernel_spmd

F32 = mybir.dt.float32
BF16 = mybir.dt.bfloat16

B, S, D, H, K = 2, 2048, 1024, 16, 64
NH = 4            # heads per core
HK = NH * K       # 256
NCORES = 8
DC = D // 128     # 8 contraction chunks
NEG = -10000.0
FQ = 512          # f-quarter width
NQ = S // FQ      # 4 f-quarters


def build_nc(t_tiles: int):
    """Per-core Bass graph. t_tiles = number of 128-row key tiles after
    host-side compaction of masked-out keys."""
    T = t_tiles * 128
    tchunks = [(c, min(512, T - c)) for c in range(0, T, 512)]

    nc = bacc.Bacc("TRN2", target_bir_lowering=False, debug=False,
                   num_devices=NCORES)

    xf_d = nc.dram_tensor("xf", [D, S], BF16, kind="ExternalInput").ap()
    xt_d = nc.dram_tensor("xt", [D, T], BF16, kind="ExternalInput").ap()
    wq_d = nc.dram_tensor("wq", [D, HK], BF16, kind="ExternalInput").ap()
    wk_d = nc.dram_tensor("wk", [D, HK], BF16, kind="ExternalInput").ap()
    wv_d = nc.dram_tensor("wv", [D, HK], BF16, kind="ExternalInput").ap()
    # bias columns: [bq0 bq1 bk0 bk1] (hk-tile halves of bq / bk)
    bias_d = nc.dram_tensor("bias", [128, 4], F32, kind="ExternalInput").ap()
    # additive key mask for the LAST key tile only (all other tiles are
    # fully unmasked after compaction)
    mask_d = nc.dram_tensor("mask", [128, 1], F32, kind="ExternalInput").ap()
    # unnormalized: per head, row 0 = softmax denominators, rows 1..64 = C^T
    out_d = nc.dram_tensor("out", [NH, K + 1, S], F32,
                           kind="ExternalOutput").ap()

    with tile.TileContext(nc) as tc:
        with (
            tc.sbuf_pool(name="const", bufs=1) as const_pool,
            tc.sbuf_pool(name="persist", bufs=1) as persist_pool,
            tc.psum_pool(name="proj", bufs=2) as proj_pool,
            tc.psum_pool(name="cacc", bufs=2) as cacc_pool,
            tc.sbuf_pool(name="et", bufs=t_tiles + 2) as et_pool,
            tc.sbuf_pool(name="ct", bufs=4) as ct_pool,
        ):
            bias_sb = const_pool.tile([128, 4], F32)
            mask_sb = const_pool.tile([128, 1], F32)
            warm_sb = const_pool.tile([128, 512], BF16)

            # one tile per DMA so consumers wait on exactly their chunk
            xf_t = [[persist_pool.tile([128, S // 2], BF16,
                                       name=f"xf{d}_{fh}")
                     for fh in range(2)] for d in range(DC)]
            xt_t = [persist_pool.tile([128, T], BF16, name=f"xt{d}")
                    for d in range(DC)]
            wq_sb = persist_pool.tile([128, DC * HK], BF16, name="wq")
            wk_sb = persist_pool.tile([128, DC * HK], BF16, name="wk")
            wv_sb = persist_pool.tile([128, DC * HK], BF16, name="wv")
            wq_v = wq_sb.rearrange("p (c s) -> p c s", s=HK)
            wk_v = wk_sb.rearrange("p (c s) -> p c s", s=HK)
            wv_v = wv_sb.rearrange("p (c s) -> p c s", s=HK)

            qt_sb = [persist_pool.tile([128, S], BF16, name=f"qt{i}")
                     for i in range(2)]
            kt_sb = [persist_pool.tile([128, T], BF16, name=f"kt{i}")
                     for i in range(2)]
            # V with a leading ones column per head: [1|V_h0|1|V_h1|...]
            v_sb = [persist_pool.tile([128, NH * 65], BF16, name=f"v{i}")
                    for i in range(t_tiles)]
            nc.vector.memset(warm_sb[:], 0.0)
            for i in range(t_tiles):
                nc.vector.memset(
                    v_sb[i].rearrange("p (h c) -> p h c", c=65)[:, :, 0:1],
                    1.0)

            # ---------------- input DMAs ----------------
            # consts first (they gate drains), then weights, then
            # interleaved xt / xf-fh0 d-chunks; xf-fh1 last.
            xt_src = xt_d.rearrange("(c p) s -> p c s", p=128)
            xf_src = xf_d.rearrange("(c p) s -> p c s", p=128)
            nc.scalar.dma_start(bias_sb[:], bias_d[:])
            nc.scalar.dma_start(mask_sb[:], mask_d[:])
            nc.scalar.dma_start(
                wq_sb.rearrange("p (c s) -> p c s", s=HK),
                wq_d.rearrange("(c p) s -> p c s", p=128))
            nc.scalar.dma_start(
                wk_sb.rearrange("p (c s) -> p c s", s=HK),
                wk_d.rearrange("(c p) s -> p c s", p=128))
            for d in range(DC):
                nc.sync.dma_start(xt_t[d][:], xt_src[:, d, :])
                nc.gpsimd.dma_start(xf_t[d][0][:], xf_src[:, d, 0:S // 2])
            nc.scalar.dma_start(
                wv_sb.rearrange("p (c s) -> p c s", s=HK),
                wv_d.rearrange("(c p) s -> p c s", p=128))
            for d in range(DC):
                nc.gpsimd.dma_start(xf_t[d][1][:], xf_src[:, d, S // 2:])

            def q_proj(hk, fh, pool, drain_eng):
                """QT[hk][:, fh*1024:+1024] <- sum_d wq_d.T @ xf_d, + bq."""
                ps = [pool.tile([128, 512], F32, tag="pp",
                                name=f"qp{hk}_{fh}_{s}")
                      for s in range(2)]
                for d in range(DC):
                    lhs = wq_v[:, d, 128 * hk:128 * (hk + 1)]
                    for s in range(2):
                        nc.tensor.matmul(ps[s][:], lhs,
                                         xf_t[d][fh][:, 512 * s:512 * (s + 1)],
                                         start=(d == 0), stop=(d == DC - 1))
                for s in range(2):
                    c0 = fh * 1024 + 512 * s
                    if drain_eng == "scalar":
                        nc.scalar.add(qt_sb[hk][:, c0:c0 + 512], ps[s][:],
                                      bias_sb[:, hk:hk + 1])
                    else:
                        nc.vector.tensor_scalar_add(
                            qt_sb[hk][:, c0:c0 + 512], ps[s][:],
                            bias_sb[:, hk:hk + 1])

            def k_chunk(hk, s, pool, drain_eng, interleave=None):
                """One T-chunk of KT[hk] <- sum_d wk_d.T @ xt_d, + bk."""
                c0, w = tchunks[s]
                pst = pool.tile([128, w], F32, tag="pp", name=f"kp{hk}_{s}")
                for d in range(DC):
                    nc.tensor.matmul(
                        pst[:], wk_v[:, d, 128 * hk:128 * (hk + 1)],
                        xt_t[d][:, c0:c0 + w],
                        start=(d == 0), stop=(d == DC - 1))
                    if interleave:
                        interleave[d]()
                if drain_eng == "scalar":
                    nc.scalar.add(kt_sb[hk][:, c0:c0 + w], pst[:],
                                  bias_sb[:, 2 + hk:3 + hk])
                else:
                    nc.vector.tensor_scalar_add(
                        kt_sb[hk][:, c0:c0 + w], pst[:],
                        bias_sb[:, 2 + hk:3 + hk])

            def v_proj(t):
                """v_sb[t][:, h*65+1 : h*65+65] <- (xt tile t).T @ wv.
                bv is added on the host (exact: probs sum to 1)."""
                ps = proj_pool.tile([128, HK], F32, tag="pp", name=f"vp{t}")
                for d in range(DC):
                    nc.tensor.matmul(ps[:],
                                     xt_t[d][:, 128 * t:128 * (t + 1)],
                                     wv_v[:, d, :],
                                     start=(d == 0), stop=(d == DC - 1))
                nc.vector.tensor_copy(
                    v_sb[t].rearrange("p (h c) -> p h c", c=65)[:, :, 1:65],
                    ps.rearrange("p (h c) -> p h c", c=64)[:, :, :])

            def attention(hk, fq, lt_pool, defer_ctx_from=None):
                """Both heads of pair hk on f-quarter fq: concurrent
                row-tiled logits -> one exp ACT -> per-head contexts.
                Context matmuls for t >= defer_ctx_from are returned as
                closures (issue them AFTER the v_proj writes)."""
                if defer_ctx_from is None:
                    defer_ctx_from = t_tiles
                hA, hB = 2 * hk, 2 * hk + 1
                c0 = FQ * fq
                caccs = [cacc_pool.tile([K + 1, FQ], F32, tag="cacc",
                                        name=f"cacc{hk}_{fq}_{i}")
                         for i in range(2)]
                deferred = []
                for t in range(t_tiles):
                    lt = lt_pool.tile([128, 2 * FQ], F32, tag="lt",
                                      name=f"lt{hk}_{fq}_{t}")
                    for i, zo in ((0, 0), (1, 64)):
                        nc.tensor.matmul(
                            lt[:, FQ * i:FQ * (i + 1)],
                            kt_sb[hk][zo:zo + 64, 128 * t:128 * (t + 1)],
                            qt_sb[hk][zo:zo + 64, c0:c0 + FQ],
                            start=True, stop=True)
                    et = et_pool.tile([128, 2 * FQ], BF16, tag="et",
                                      name=f"et{hk}_{fq}_{t}")
                    nc.scalar.activation(
                        et[:], lt[:], mybir.ActivationFunctionType.Exp,
                        bias=(mask_sb[:, 0:1] if t == t_tiles - 1 else 0.0),
                        scale=0.125)

                    def ctx(t=t, et=et):
                        for i, h in ((0, hA), (1, hB)):
                            nc.tensor.matmul(
                                caccs[i][:],
                                v_sb[t][:, 65 * h:65 * (h + 1)],
                                et[:, FQ * i:FQ * (i + 1)],
                                start=(t == 0), stop=(t == t_tiles - 1),
                                skip_group_check=True)

                    if t >= defer_ctx_from:
                        deferred.append(ctx)
                    else:
                        ctx()

                def tail():
                    for i, h in ((0, hA), (1, hB)):
                        ct = ct_pool.tile([K + 1, FQ], F32, tag="ct",
                                          name=f"ct{hk}_{fq}_{i}")
                        nc.vector.tensor_copy(ct[:], caccs[i][:])
                        nc.sync.dma_start(out_d[h][:, c0:c0 + FQ], ct[:])

                deferred.append(tail)
                return deferred

            def run(ops):
                for op in ops:
                    op()

            # ---------------- schedule ----------------
            # PE warmup: keep the HAM clock-gate hot until real work lands.
            with tc.psum_pool(name="warm", bufs=1) as warm_pool:
                wps = warm_pool.tile([128, 512], F32, tag="wp", name="warm")
                for i in range(32):
                    nc.tensor.matmul(wps[:], warm_sb[:, 0:128], warm_sb[:],
                                     start=True, stop=True)

            # critical path: Q(pair0, fh0) and K(pair0) interleaved d-wise
            # in a dedicated 4-bank pool that closes before lt opens.
            with tc.psum_pool(name="early", bufs=4) as early_pool:
                q_ps = [early_pool.tile([128, 512], F32, tag="pp",
                                        name=f"qp0_0_{s}")
                        for s in range(2)]

                def qcb(d):
                    lhs = wq_v[:, d, 0:128]
                    for s in range(2):
                        nc.tensor.matmul(
                            q_ps[s][:], lhs,
                            xf_t[d][0][:, 512 * s:512 * (s + 1)],
                            start=(d == 0), stop=(d == DC - 1))

                qcbs = [(lambda d=d: qcb(d)) for d in range(DC)]
                k_chunk(0, 0, early_pool, "scalar", interleave=qcbs)
                for s in range(2):
                    nc.vector.tensor_scalar_add(
                        qt_sb[0][:, 512 * s:512 * (s + 1)], q_ps[s][:],
                        bias_sb[:, 0:1])
                if len(tchunks) > 1:
                    k_chunk(0, 1, early_pool, "scalar")

            with tc.psum_pool(name="lt", bufs=2) as lt_pool:
                for s in range(2, len(tchunks)):
                    k_chunk(0, s, proj_pool, "scalar")
                v_proj(0)
                # pair0/fq0: logits+exp start immediately; contexts for
                # t>=1 are deferred until after the v_proj writes.
                pend = attention(0, 0, lt_pool, defer_ctx_from=1)
                for t in range(1, t_tiles):
                    v_proj(t)
                run(pend)
                run(attention(0, 1, lt_pool))
                for s in range(len(tchunks)):
                    k_chunk(1, s, proj_pool, "vector")
                q_proj(0, 1, proj_pool, "vector")
                run(attention(0, 2, lt_pool))
                q_proj(1, 0, proj_pool, "vector")
                run(attention(0, 3, lt_pool))
                q_proj(1, 1, proj_pool, "vector")
                run(attention(1, 0, lt_pool))
                run(attention(1, 1, lt_pool))
                run(attention(1, 2, lt_pool))
                run(attention(1, 3, lt_pool))

    nc.compile()
    return nc


_NC_CACHE = {}


def _get_nc(t_tiles: int):
    if t_tiles not in _NC_CACHE:
        _NC_CACHE[t_tiles] = build_nc(t_tiles)
    return _NC_CACHE[t_tiles]


def kernel(from_tensor, to_tensor, attention_mask, Wq, bq, Wk, bk, Wv, bv):
    from_tensor = np.asarray(from_tensor, dtype=np.float32)
    to_tensor = np.asarray(to_tensor, dtype=np.float32)
    attention_mask = np.asarray(attention_mask)
    Wq = np.asarray(Wq, dtype=np.float32)
    Wk = np.asarray(Wk, dtype=np.float32)
    Wv = np.asarray(Wv, dtype=np.float32)
    bq = np.asarray(bq, dtype=np.float32)
    bk = np.asarray(bk, dtype=np.float32)
    bv = np.asarray(bv, dtype=np.float32)

    # compact away masked-out keys (they contribute exactly 0 to the
    # context); pad to a 128 multiple and re-mask the padding tail.
    mask_np = attention_mask.astype(np.int32)
    idxs = [np.nonzero(mask_np[b])[0] for b in range(B)]
    t_eff = max(1, max(len(ix) for ix in idxs))
    T_pad = min(S, ((t_eff + 127) // 128) * 128)
    t_tiles = T_pad // 128
    nc = _get_nc(t_tiles)

    xt_c = np.zeros((B, D, T_pad), dtype=np.float32)
    maskadd = np.full((B, T_pad), NEG, dtype=np.float32)
    for b in range(B):
        ix = idxs[b]
        xt_c[b, :, :len(ix)] = to_tensor[b].T[:, ix]
        maskadd[b, :len(ix)] = 0.0

    in_maps = []
    for c in range(NCORES):
        b, g = c // 4, c % 4
        hs = slice(NH * g, NH * (g + 1))
        wq = np.ascontiguousarray(Wq[:, hs, :].reshape(D, HK))
        wk = np.ascontiguousarray(Wk[:, hs, :].reshape(D, HK))
        wv = np.ascontiguousarray(Wv[:, hs, :].reshape(D, HK))
        bias = np.stack([
            bq[hs].reshape(HK)[:128], bq[hs].reshape(HK)[128:],
            bk[hs].reshape(HK)[:128], bk[hs].reshape(HK)[128:],
        ], axis=1)
        in_maps.append({
            "xf": np.ascontiguousarray(from_tensor[b].T
                                       .astype(ml_dtypes.bfloat16)),
            "xt": np.ascontiguousarray(xt_c[b].astype(ml_dtypes.bfloat16)),
            "wq": wq.astype(ml_dtypes.bfloat16),
            "wk": wk.astype(ml_dtypes.bfloat16),
            "wv": wv.astype(ml_dtypes.bfloat16),
            "bias": np.ascontiguousarray(bias),
            "mask": np.ascontiguousarray(
                maskadd[b][(t_tiles - 1) * 128:].reshape(128, 1)),
        })

    global _LAST_IN_MAPS, _LAST_T_TILES
    _LAST_IN_MAPS = in_maps
    _LAST_T_TILES = t_tiles
    try:
        res = run_bass_kernel_spmd(nc, in_maps, core_ids=list(range(NCORES)))
    except Exception:
        # the axon terminal occasionally reports the device unrecoverable;
        # a reset + retry clears it
        try:
            import ctypes

            lib = ctypes.CDLL("/opt/axon/libaxon_pjrt.so")
            lib.axon_reset.restype = ctypes.c_int64
            lib.axon_reset()
        except Exception:
            pass
        res = run_bass_kernel_spmd(nc, in_maps, core_ids=list(range(NCORES)))

    out = np.empty((B, S, H, K), dtype=np.float32)
    for c in range(NCORES):
        b, g = c // 4, c % 4
        o = res.results[c]["out"]          # [NH, 65, S]
        ctx = o[:, 1:, :] / o[:, 0:1, :]   # normalize by denominators
        # [NH, K, S] -> [S, NH, K], plus bv
        out[b, :, NH * g:NH * (g + 1), :] = \
            ctx.transpose(2, 0, 1) + bv[NH * g:NH * (g + 1)][None]
    return out
